# revision 10
# baseline (speedup 1.0000x reference)
"""Trainium2 Bass kernel for HHNodeMP message passing.

Reference computation (per row n of N=100000, d=256):
    node_fea = cur @ Wn
    spa_fea  = spa @ Ws
    tmp_fea  = tmp @ Wt
    s[n] = (spa_fea[n] . node_fea[n]) / 16
    t[n] = (tmp_fea[n] . node_fea[n]) / 16
    out  = relu((s*spa_fea + t*tmp_fea) @ theta_w.T + theta_b)

Algebraic restructuring (weight products precomputed on host):
    s[n] = rowsum((spa @ (Ws Wn^T / 16)) * cur)
    t[n] = rowsum((tmp @ (Wt Wn^T / 16)) * cur)
    out  = relu(s * (spa @ Ws th^T) + t * (tmp @ Wt th^T) + b)
On-device: TWO matmuls per 128-row tile with concatenated [256, 512]
weights:  spa @ [Ms | Wsp] -> [q_s | g1],  tmp @ [Mt | Wtp] -> [q_t | g2].

Perf design:
  - All activations cast to bf16 on host (halves HBM traffic; PE runs
    1 cycle/row for bf16 and f32r alike, so no PE cost).
  - spa/tmp are transposed on the HOST into [128, 2, ROWS] (k-major)
    layout, eliminating all on-device PE transposes + psum copies.
  - cur and out use partition-major [128, TILES, 256] host layouts so
    every DMA is a large contiguous run per partition.
  - Group DMAs: G=14 tiles per DMA (7KB/partition contiguous).
  - Vector work balanced across engines per tile:
      DVE : dot_s (STT+accum)  + pair-batched add (pre+h2)
      Pool: dot_t (STT+accum)  + pre = s*g1 + b
      Act : h2 = t*g2 (Copy with per-partition scale) + pair-batched Relu
  - PSUM: 2 pools x 4 banks -> 4-tile-deep matmul pipelining keeps the
    PE continuously busy (full 2.4 GHz p-state).

Sharding: row-parallel across 8 NeuronCores, 12544 rows/core (zero-
padded from 12500), weights replicated. No communication.
"""

import sys

import numpy as np

sys.path.insert(0, "/opt/trn_rl_repo")

import ml_dtypes  # noqa: E402

import concourse.bass as bass  # noqa: E402
import concourse.mybir as mybir  # noqa: E402
import concourse.tile as tile  # noqa: E402
from concourse import bacc  # noqa: E402
from concourse.bass_utils import run_bass_kernel_spmd  # noqa: E402

N = 100000
D = 256
N_CORES = 8
ROWS_RAW = N // N_CORES            # 12500
TILES = (ROWS_RAW + 127) // 128    # 98
ROWS = TILES * 128                 # 12544
G = 14                             # tiles per DMA group
GROUPS = TILES // G                # 7
F32 = mybir.dt.float32
BF16 = mybir.dt.bfloat16
BF16_NP = ml_dtypes.bfloat16

_CACHED_NC = None


def _build_nc() -> bass.Bass:
    nc = bacc.Bacc("TRN2", target_bir_lowering=False, debug=False)

    spa_d = nc.declare_dram_parameter("spaT", [128, 2, ROWS], BF16, isOutput=False)
    tmp_d = nc.declare_dram_parameter("tmpT", [128, 2, ROWS], BF16, isOutput=False)
    cur_d = nc.declare_dram_parameter("curP", [128, TILES, D], BF16, isOutput=False)
    ws_d = nc.declare_dram_parameter("w_scat", [128, 2, 2 * D], BF16, isOutput=False)
    wt_d = nc.declare_dram_parameter("w_tcat", [128, 2, 2 * D], BF16, isOutput=False)
    out_d = nc.declare_dram_parameter("out", [128, TILES, D], BF16, isOutput=True)

    MULT = mybir.AluOpType.mult
    ADD = mybir.AluOpType.add

    with tile.TileContext(nc) as tc:
        with (
            tc.tile_pool(name="const", bufs=1) as cpool,
            tc.tile_pool(name="din", bufs=3) as ipool,
            tc.tile_pool(name="dout", bufs=6) as opool,
            tc.tile_pool(name="work", bufs=6) as wpool,
            tc.tile_pool(name="ps_s", bufs=4, space="PSUM") as pspool,
            tc.tile_pool(name="ps_t", bufs=4, space="PSUM") as ptpool,
        ):
            w_s = cpool.tile([128, 2, 2 * D], BF16)
            nc.sync.dma_start(w_s[:], ws_d[:])
            w_t = cpool.tile([128, 2, 2 * D], BF16)
            nc.sync.dma_start(w_t[:], wt_d[:])

            group_sizes = [2, 4, 8] + [14] * 6
            assert sum(group_sizes) == TILES
            t0 = 0
            for gs in group_sizes:
                n0 = t0 * 128
                sa = ipool.tile([128, 2, G * 128], BF16, tag="sa")
                nc.sync.dma_start(sa[:, :, 0 : gs * 128], spa_d[:, :, n0 : n0 + gs * 128])
                ta = ipool.tile([128, 2, G * 128], BF16, tag="ta")
                nc.sync.dma_start(ta[:, :, 0 : gs * 128], tmp_d[:, :, n0 : n0 + gs * 128])
                cu = ipool.tile([128, G, D], BF16, tag="cu")
                nc.sync.dma_start(cu[:, 0:gs], cur_d[:, t0 : t0 + gs, :])

                for j in range(gs // 2):
                    # Per-pair scratch (pair = two 128-row tiles)
                    s_at = wpool.tile([128, 2], F32, tag="s_at")
                    t_at = wpool.tile([128, 2], F32, tag="t_at")
                    scr_s = wpool.tile([128, 2, D], BF16, tag="scr_s")
                    scr_t = wpool.tile([128, 2, D], BF16, tag="scr_t")
                    pre = wpool.tile([128, 2, D], BF16, tag="pre")
                    h2 = wpool.tile([128, 2, D], BF16, tag="h2")


                    for u in range(2):
                        ti = 2 * j + u
                        m0 = ti * 128
                        ps_s = pspool.tile([128, 2 * D], F32, tag="ps_s")
                        ps_t = ptpool.tile([128, 2 * D], F32, tag="ps_t")
                        for kc in range(2):
                            nc.tensor.matmul(
                                ps_s[:],
                                sa[:, kc, m0 : m0 + 128],
                                w_s[:, kc],
                                start=(kc == 0),
                                stop=(kc == 1),
                            )
                        for kc in range(2):
                            nc.tensor.matmul(
                                ps_t[:],
                                ta[:, kc, m0 : m0 + 128],
                                w_t[:, kc],
                                start=(kc == 0),
                                stop=(kc == 1),
                            )

                        # s = rowsum(q_s * cur)   (1/16 folded into Ms)
                        nc.vector.scalar_tensor_tensor(
                            out=scr_s[:, u],
                            in0=ps_s[:, 0:D],
                            scalar=1.0,
                            in1=cu[:, ti],
                            op0=MULT,
                            op1=MULT,
                            accum_out=s_at[:, u : u + 1],
                        )
                        nc.vector.scalar_tensor_tensor(
                            out=scr_t[:, u],
                            in0=ps_t[:, 0:D],
                            scalar=1.0,
                            in1=cu[:, ti],
                            op0=MULT,
                            op1=MULT,
                            accum_out=t_at[:, u : u + 1],
                        )
                        # pre = s*g1   (Act drains psum with per-partition scale)
                        nc.scalar.activation(
                            pre[:, u],
                            ps_s[:, D : 2 * D],
                            mybir.ActivationFunctionType.Copy,
                            bias=0.0,
                            scale=s_at[:, u : u + 1],
                        )
                        # h2 = t*g2
                        nc.scalar.activation(
                            h2[:, u],
                            ps_t[:, D : 2 * D],
                            mybir.ActivationFunctionType.Copy,
                            bias=0.0,
                            scale=t_at[:, u : u + 1],
                        )

                    # Pair-batched SBUF finish (Pool): out = pre + h2
                    # (bias add + relu happen on the host)
                    ot = opool.tile([128, 2, D], BF16, tag="ot")
                    nc.gpsimd.tensor_add(ot[:], pre[:], h2[:])
                    tp = t0 + 2 * j
                    nc.sync.dma_start(out_d[:, tp : tp + 2, :], ot[:])

                t0 += gs

    nc.compile()
    return nc


def _get_nc() -> bass.Bass:
    global _CACHED_NC
    if _CACHED_NC is None:
        _CACHED_NC = _build_nc()
    return _CACHED_NC


def _prep_inputs(
    cur, spatial_hyperedge_emb, temporal_hyperedge_emb,
    node_proj, spatial_edge_proj, temporal_edge_proj, theta_w, theta_b,
):
    cur = np.ascontiguousarray(cur, np.float32)
    spa = np.ascontiguousarray(spatial_hyperedge_emb, np.float32)
    tmp = np.ascontiguousarray(temporal_hyperedge_emb, np.float32)
    wn = np.asarray(node_proj, np.float64)
    ws = np.asarray(spatial_edge_proj, np.float64)
    wt = np.asarray(temporal_edge_proj, np.float64)
    th = np.asarray(theta_w, np.float64)
    b = np.asarray(theta_b, np.float32)

    w_scat = np.concatenate([(ws @ wn.T) / 16.0, ws @ th.T], axis=1)
    w_tcat = np.concatenate([(wt @ wn.T) / 16.0, wt @ th.T], axis=1)
    # [256, 512] -> [128, 2, 512]: [p, kc, f] = w[kc*128+p, f]
    w_scat = w_scat.reshape(2, 128, 2 * D).transpose(1, 0, 2).astype(BF16_NP)
    w_tcat = w_tcat.reshape(2, 128, 2 * D).transpose(1, 0, 2).astype(BF16_NP)
    pad = N_CORES * ROWS - N

    def shard(x):
        x = np.concatenate([x, np.zeros((pad, D), np.float32)], axis=0)
        return x.reshape(N_CORES, ROWS, D)

    cur_s, spa_s, tmp_s = shard(cur), shard(spa), shard(tmp)
    # k-major transpose: [ROWS, 256] -> [128, 2, ROWS]: [p, kc, n] = x[n, kc*128+p]
    spa_T = np.ascontiguousarray(
        spa_s.reshape(N_CORES, ROWS, 2, 128).transpose(0, 3, 2, 1)
    ).astype(BF16_NP)
    tmp_T = np.ascontiguousarray(
        tmp_s.reshape(N_CORES, ROWS, 2, 128).transpose(0, 3, 2, 1)
    ).astype(BF16_NP)
    # partition-major: [ROWS, 256] -> [128, TILES, 256]: [p, t, f] = x[t*128+p, f]
    cur_P = np.ascontiguousarray(
        cur_s.reshape(N_CORES, TILES, 128, D).transpose(0, 2, 1, 3)
    ).astype(BF16_NP)

    in_maps = []
    for c in range(N_CORES):
        in_maps.append(
            {
                "spaT": np.ascontiguousarray(spa_T[c]),
                "tmpT": np.ascontiguousarray(tmp_T[c]),
                "curP": np.ascontiguousarray(cur_P[c]),
                "w_scat": w_scat,
                "w_tcat": w_tcat,
            }
        )
    return in_maps


def kernel(**inputs) -> np.ndarray:
    in_maps = _prep_inputs(**inputs)
    nc = _get_nc()
    res = run_bass_kernel_spmd(nc, in_maps, list(range(N_CORES)))
    b = np.asarray(inputs["theta_b"], np.float32)
    # out: [128, TILES, 256] partition-major bf16 -> [ROWS, 256] f32;
    # bias + relu applied on host (elementwise epilogue)
    outs = [
        np.asarray(res.results[c]["out"])
        .transpose(1, 0, 2)
        .reshape(ROWS, D)
        .astype(np.float32)
        for c in range(N_CORES)
    ]
    z = np.concatenate(outs, axis=0)[:N] + b
    return np.ascontiguousarray(np.maximum(z, 0.0))


# revision 11
# speedup vs baseline: 1.0855x; 1.0855x over previous
"""Trainium2 Bass kernel for HHNodeMP message passing.

Reference computation (per row n of N=100000, d=256):
    node_fea = cur @ Wn
    spa_fea  = spa @ Ws
    tmp_fea  = tmp @ Wt
    s[n] = (spa_fea[n] . node_fea[n]) / 16
    t[n] = (tmp_fea[n] . node_fea[n]) / 16
    out  = relu((s*spa_fea + t*tmp_fea) @ theta_w.T + theta_b)

Algebraic restructuring (weight products precomputed on host):
    s[n] = rowsum((spa @ (Ws Wn^T / 16)) * cur)
    t[n] = rowsum((tmp @ (Wt Wn^T / 16)) * cur)
    out  = relu(s * (spa @ Ws th^T) + t * (tmp @ Wt th^T) + b)
On-device: TWO matmuls per 128-row tile with concatenated [256, 512]
weights:  spa @ [Ms | Wsp] -> [q_s | g1],  tmp @ [Mt | Wtp] -> [q_t | g2].

Perf design:
  - All activations cast to bf16 on host (halves HBM traffic; PE runs
    1 cycle/row for bf16 and f32r alike, so no PE cost).
  - spa/tmp are transposed on the HOST into [128, 2, ROWS] (k-major)
    layout, eliminating all on-device PE transposes + psum copies.
  - cur and out use partition-major [128, TILES, 256] host layouts so
    every DMA is a large contiguous run per partition.
  - Group DMAs: G=14 tiles per DMA (7KB/partition contiguous).
  - Vector work balanced across engines per tile:
      DVE : dot_s (STT+accum)  + pair-batched add (pre+h2)
      Pool: dot_t (STT+accum)  + pre = s*g1 + b
      Act : h2 = t*g2 (Copy with per-partition scale) + pair-batched Relu
  - PSUM: 2 pools x 4 banks -> 4-tile-deep matmul pipelining keeps the
    PE continuously busy (full 2.4 GHz p-state).

Sharding: row-parallel across 8 NeuronCores, 12544 rows/core (zero-
padded from 12500), weights replicated. No communication.
"""

import sys

import numpy as np

sys.path.insert(0, "/opt/trn_rl_repo")

import ml_dtypes  # noqa: E402

import concourse.bass as bass  # noqa: E402
import concourse.mybir as mybir  # noqa: E402
import concourse.tile as tile  # noqa: E402
from concourse import bacc  # noqa: E402
from concourse.bass_utils import run_bass_kernel_spmd  # noqa: E402

N = 100000
D = 256
N_CORES = 8
ROWS_RAW = N // N_CORES            # 12500
TILES = (ROWS_RAW + 127) // 128    # 98
ROWS = TILES * 128                 # 12544
G = 14                             # tiles per DMA group
GROUPS = TILES // G                # 7
F32 = mybir.dt.float32
BF16 = mybir.dt.bfloat16
BF16_NP = ml_dtypes.bfloat16

_CACHED_NC = None


def _build_nc() -> bass.Bass:
    nc = bacc.Bacc("TRN2", target_bir_lowering=False, debug=False)

    spa_d = nc.declare_dram_parameter("spaT", [128, 2, ROWS], BF16, isOutput=False)
    tmp_d = nc.declare_dram_parameter("tmpT", [128, 2, ROWS], BF16, isOutput=False)
    cur_d = nc.declare_dram_parameter("curP", [128, TILES, D], BF16, isOutput=False)
    ws_d = nc.declare_dram_parameter("w_scat", [128, 2, 2 * D], BF16, isOutput=False)
    wt_d = nc.declare_dram_parameter("w_tcat", [128, 2, 2 * D], BF16, isOutput=False)
    out_d = nc.declare_dram_parameter("out", [128, TILES, D], BF16, isOutput=True)

    MULT = mybir.AluOpType.mult
    ADD = mybir.AluOpType.add

    with tile.TileContext(nc) as tc:
        with (
            tc.tile_pool(name="const", bufs=1) as cpool,
            tc.tile_pool(name="din", bufs=3) as ipool,
            tc.tile_pool(name="dout", bufs=3) as opool,
            tc.tile_pool(name="work", bufs=6) as wpool,
            tc.tile_pool(name="ps_s", bufs=4, space="PSUM") as pspool,
            tc.tile_pool(name="ps_t", bufs=4, space="PSUM") as ptpool,
        ):
            w_s = cpool.tile([128, 2, 2 * D], BF16)
            nc.sync.dma_start(w_s[:], ws_d[:])
            w_t = cpool.tile([128, 2, 2 * D], BF16)
            nc.sync.dma_start(w_t[:], wt_d[:])

            group_sizes = [2, 4, 8] + [14] * 5 + [8, 4, 2]
            assert sum(group_sizes) == TILES
            t0 = 0
            for gs in group_sizes:
                n0 = t0 * 128
                sa = ipool.tile([128, 2, G * 128], BF16, tag="sa")
                nc.sync.dma_start(sa[:, :, 0 : gs * 128], spa_d[:, :, n0 : n0 + gs * 128])
                ta = ipool.tile([128, 2, G * 128], BF16, tag="ta")
                nc.sync.dma_start(ta[:, :, 0 : gs * 128], tmp_d[:, :, n0 : n0 + gs * 128])
                cu = ipool.tile([128, G, D], BF16, tag="cu")
                nc.sync.dma_start(cu[:, 0:gs], cur_d[:, t0 : t0 + gs, :])
                ot = opool.tile([128, G, D], BF16, tag="ot")

                for j in range(gs // 2):
                    # Per-pair scratch (pair = two 128-row tiles)
                    s_at = wpool.tile([128, 2], F32, tag="s_at")
                    t_at = wpool.tile([128, 2], F32, tag="t_at")
                    scr_s = wpool.tile([128, 2, D], BF16, tag="scr_s")
                    scr_t = wpool.tile([128, 2, D], BF16, tag="scr_t")
                    pre = wpool.tile([128, 2, D], BF16, tag="pre")
                    h2 = wpool.tile([128, 2, D], BF16, tag="h2")


                    for u in range(2):
                        ti = 2 * j + u
                        m0 = ti * 128
                        ps_s = pspool.tile([128, 2 * D], F32, tag="ps_s")
                        ps_t = ptpool.tile([128, 2 * D], F32, tag="ps_t")
                        for kc in range(2):
                            nc.tensor.matmul(
                                ps_s[:],
                                sa[:, kc, m0 : m0 + 128],
                                w_s[:, kc],
                                start=(kc == 0),
                                stop=(kc == 1),
                            )
                        for kc in range(2):
                            nc.tensor.matmul(
                                ps_t[:],
                                ta[:, kc, m0 : m0 + 128],
                                w_t[:, kc],
                                start=(kc == 0),
                                stop=(kc == 1),
                            )

                        # s = rowsum(q_s * cur)   (1/16 folded into Ms)
                        nc.vector.scalar_tensor_tensor(
                            out=scr_s[:, u],
                            in0=ps_s[:, 0:D],
                            scalar=1.0,
                            in1=cu[:, ti],
                            op0=MULT,
                            op1=MULT,
                            accum_out=s_at[:, u : u + 1],
                        )
                        nc.vector.scalar_tensor_tensor(
                            out=scr_t[:, u],
                            in0=ps_t[:, 0:D],
                            scalar=1.0,
                            in1=cu[:, ti],
                            op0=MULT,
                            op1=MULT,
                            accum_out=t_at[:, u : u + 1],
                        )
                        # pre = s*g1   (Act drains psum with per-partition scale)
                        nc.scalar.activation(
                            pre[:, u],
                            ps_s[:, D : 2 * D],
                            mybir.ActivationFunctionType.Copy,
                            bias=0.0,
                            scale=s_at[:, u : u + 1],
                        )
                        # h2 = t*g2
                        nc.scalar.activation(
                            h2[:, u],
                            ps_t[:, D : 2 * D],
                            mybir.ActivationFunctionType.Copy,
                            bias=0.0,
                            scale=t_at[:, u : u + 1],
                        )

                    # Pair-batched SBUF finish (Pool): out = pre + h2
                    # (bias add + relu happen on the host)
                    nc.gpsimd.tensor_add(ot[:, 2 * j : 2 * j + 2], pre[:], h2[:])

                nc.sync.dma_start(out_d[:, t0 : t0 + gs, :], ot[:, 0:gs])
                t0 += gs

    nc.compile()
    return nc


def _get_nc() -> bass.Bass:
    global _CACHED_NC
    if _CACHED_NC is None:
        _CACHED_NC = _build_nc()
    return _CACHED_NC


def _prep_inputs(
    cur, spatial_hyperedge_emb, temporal_hyperedge_emb,
    node_proj, spatial_edge_proj, temporal_edge_proj, theta_w, theta_b,
):
    cur = np.ascontiguousarray(cur, np.float32)
    spa = np.ascontiguousarray(spatial_hyperedge_emb, np.float32)
    tmp = np.ascontiguousarray(temporal_hyperedge_emb, np.float32)
    wn = np.asarray(node_proj, np.float64)
    ws = np.asarray(spatial_edge_proj, np.float64)
    wt = np.asarray(temporal_edge_proj, np.float64)
    th = np.asarray(theta_w, np.float64)
    b = np.asarray(theta_b, np.float32)

    w_scat = np.concatenate([(ws @ wn.T) / 16.0, ws @ th.T], axis=1)
    w_tcat = np.concatenate([(wt @ wn.T) / 16.0, wt @ th.T], axis=1)
    # [256, 512] -> [128, 2, 512]: [p, kc, f] = w[kc*128+p, f]
    w_scat = w_scat.reshape(2, 128, 2 * D).transpose(1, 0, 2).astype(BF16_NP)
    w_tcat = w_tcat.reshape(2, 128, 2 * D).transpose(1, 0, 2).astype(BF16_NP)
    pad = N_CORES * ROWS - N

    def shard(x):
        x = np.concatenate([x, np.zeros((pad, D), np.float32)], axis=0)
        return x.reshape(N_CORES, ROWS, D)

    cur_s, spa_s, tmp_s = shard(cur), shard(spa), shard(tmp)
    # k-major transpose: [ROWS, 256] -> [128, 2, ROWS]: [p, kc, n] = x[n, kc*128+p]
    spa_T = np.ascontiguousarray(
        spa_s.reshape(N_CORES, ROWS, 2, 128).transpose(0, 3, 2, 1)
    ).astype(BF16_NP)
    tmp_T = np.ascontiguousarray(
        tmp_s.reshape(N_CORES, ROWS, 2, 128).transpose(0, 3, 2, 1)
    ).astype(BF16_NP)
    # partition-major: [ROWS, 256] -> [128, TILES, 256]: [p, t, f] = x[t*128+p, f]
    cur_P = np.ascontiguousarray(
        cur_s.reshape(N_CORES, TILES, 128, D).transpose(0, 2, 1, 3)
    ).astype(BF16_NP)

    in_maps = []
    for c in range(N_CORES):
        in_maps.append(
            {
                "spaT": np.ascontiguousarray(spa_T[c]),
                "tmpT": np.ascontiguousarray(tmp_T[c]),
                "curP": np.ascontiguousarray(cur_P[c]),
                "w_scat": w_scat,
                "w_tcat": w_tcat,
            }
        )
    return in_maps


def kernel(**inputs) -> np.ndarray:
    in_maps = _prep_inputs(**inputs)
    nc = _get_nc()
    res = run_bass_kernel_spmd(nc, in_maps, list(range(N_CORES)))
    b = np.asarray(inputs["theta_b"], np.float32)
    # out: [128, TILES, 256] partition-major bf16 -> [ROWS, 256] f32;
    # bias + relu applied on host (elementwise epilogue)
    outs = [
        np.asarray(res.results[c]["out"])
        .transpose(1, 0, 2)
        .reshape(ROWS, D)
        .astype(np.float32)
        for c in range(N_CORES)
    ]
    z = np.concatenate(outs, axis=0)[:N] + b
    return np.ascontiguousarray(np.maximum(z, 0.0))


# revision 12
# speedup vs baseline: 1.1081x; 1.0208x over previous
"""Trainium2 Bass kernel for HHNodeMP message passing.

Reference computation (per row n of N=100000, d=256):
    node_fea = cur @ Wn
    spa_fea  = spa @ Ws
    tmp_fea  = tmp @ Wt
    s[n] = (spa_fea[n] . node_fea[n]) / 16
    t[n] = (tmp_fea[n] . node_fea[n]) / 16
    out  = relu((s*spa_fea + t*tmp_fea) @ theta_w.T + theta_b)

Algebraic restructuring (weight products precomputed on host):
    s[n] = rowsum((spa @ (Ws Wn^T / 16)) * cur)
    t[n] = rowsum((tmp @ (Wt Wn^T / 16)) * cur)
    out  = relu(s * (spa @ Ws th^T) + t * (tmp @ Wt th^T) + b)
On-device: TWO matmuls per 128-row tile with concatenated [256, 512]
weights:  spa @ [Ms | Wsp] -> [q_s | g1],  tmp @ [Mt | Wtp] -> [q_t | g2].

Perf design:
  - All activations cast to bf16 on host (halves HBM traffic; PE runs
    1 cycle/row for bf16 and f32r alike, so no PE cost).
  - spa/tmp are transposed on the HOST into [128, 2, ROWS] (k-major)
    layout, eliminating all on-device PE transposes + psum copies.
  - cur and out use partition-major [128, TILES, 256] host layouts so
    every DMA is a large contiguous run per partition.
  - Group DMAs: G=14 tiles per DMA (7KB/partition contiguous).
  - Vector work balanced across engines per tile:
      DVE : dot_s (STT+accum)  + pair-batched add (pre+h2)
      Pool: dot_t (STT+accum)  + pre = s*g1 + b
      Act : h2 = t*g2 (Copy with per-partition scale) + pair-batched Relu
  - PSUM: 2 pools x 4 banks -> 4-tile-deep matmul pipelining keeps the
    PE continuously busy (full 2.4 GHz p-state).

Sharding: row-parallel across 8 NeuronCores, 12544 rows/core (zero-
padded from 12500), weights replicated. No communication.
"""

import sys

import numpy as np

sys.path.insert(0, "/opt/trn_rl_repo")

import ml_dtypes  # noqa: E402

import concourse.bass as bass  # noqa: E402
import concourse.mybir as mybir  # noqa: E402
import concourse.tile as tile  # noqa: E402
from concourse import bacc  # noqa: E402
from concourse.bass_utils import run_bass_kernel_spmd  # noqa: E402

N = 100000
D = 256
N_CORES = 8
ROWS_RAW = N // N_CORES            # 12500
TILES = (ROWS_RAW + 127) // 128    # 98
ROWS = TILES * 128                 # 12544
G = 14                             # tiles per DMA group
GROUPS = TILES // G                # 7
F32 = mybir.dt.float32
BF16 = mybir.dt.bfloat16
BF16_NP = ml_dtypes.bfloat16

_CACHED_NC = None


def _build_nc() -> bass.Bass:
    nc = bacc.Bacc("TRN2", target_bir_lowering=False, debug=False)

    spa_d = nc.declare_dram_parameter("spaT", [128, 2, ROWS], BF16, isOutput=False)
    tmp_d = nc.declare_dram_parameter("tmpT", [128, 2, ROWS], BF16, isOutput=False)
    cur_d = nc.declare_dram_parameter("curP", [128, TILES, D], BF16, isOutput=False)
    ws_d = nc.declare_dram_parameter("w_scat", [128, 2, 2 * D], BF16, isOutput=False)
    wt_d = nc.declare_dram_parameter("w_tcat", [128, 2, 2 * D], BF16, isOutput=False)
    out_d = nc.declare_dram_parameter("out", [128, TILES, D], BF16, isOutput=True)

    MULT = mybir.AluOpType.mult
    ADD = mybir.AluOpType.add

    with tile.TileContext(nc) as tc:
        with (
            tc.tile_pool(name="const", bufs=1) as cpool,
            tc.tile_pool(name="din", bufs=4) as ipool,
            tc.tile_pool(name="dout", bufs=3) as opool,
            tc.tile_pool(name="work", bufs=6) as wpool,
            tc.tile_pool(name="ps_s", bufs=4, space="PSUM") as pspool,
            tc.tile_pool(name="ps_t", bufs=4, space="PSUM") as ptpool,
        ):
            w_s = cpool.tile([128, 2, 2 * D], BF16)
            nc.sync.dma_start(w_s[:], ws_d[:])
            w_t = cpool.tile([128, 2, 2 * D], BF16)
            nc.sync.dma_start(w_t[:], wt_d[:])

            group_sizes = [4, 10] + [14] * 5 + [10, 4]
            assert sum(group_sizes) == TILES
            t0 = 0
            for gs in group_sizes:
                n0 = t0 * 128
                sa = ipool.tile([128, 2, G * 128], BF16, tag="sa")
                nc.sync.dma_start(sa[:, :, 0 : gs * 128], spa_d[:, :, n0 : n0 + gs * 128])
                ta = ipool.tile([128, 2, G * 128], BF16, tag="ta")
                nc.sync.dma_start(ta[:, :, 0 : gs * 128], tmp_d[:, :, n0 : n0 + gs * 128])
                cu = ipool.tile([128, G, D], BF16, tag="cu")
                nc.sync.dma_start(cu[:, 0:gs], cur_d[:, t0 : t0 + gs, :])
                ot = opool.tile([128, G, D], BF16, tag="ot")

                for j in range(gs // 2):
                    # Per-pair scratch (pair = two 128-row tiles)
                    s_at = wpool.tile([128, 2], F32, tag="s_at")
                    t_at = wpool.tile([128, 2], F32, tag="t_at")
                    scr_s = wpool.tile([128, 2, D], BF16, tag="scr_s")
                    scr_t = wpool.tile([128, 2, D], BF16, tag="scr_t")
                    pre = wpool.tile([128, 2, D], BF16, tag="pre")
                    h2 = wpool.tile([128, 2, D], BF16, tag="h2")


                    for u in range(2):
                        ti = 2 * j + u
                        m0 = ti * 128
                        ps_s = pspool.tile([128, 2 * D], F32, tag="ps_s")
                        ps_t = ptpool.tile([128, 2 * D], F32, tag="ps_t")
                        for kc in range(2):
                            nc.tensor.matmul(
                                ps_s[:],
                                sa[:, kc, m0 : m0 + 128],
                                w_s[:, kc],
                                start=(kc == 0),
                                stop=(kc == 1),
                            )
                        for kc in range(2):
                            nc.tensor.matmul(
                                ps_t[:],
                                ta[:, kc, m0 : m0 + 128],
                                w_t[:, kc],
                                start=(kc == 0),
                                stop=(kc == 1),
                            )

                        # s = rowsum(q_s * cur)   (1/16 folded into Ms)
                        nc.vector.scalar_tensor_tensor(
                            out=scr_s[:, u],
                            in0=ps_s[:, 0:D],
                            scalar=1.0,
                            in1=cu[:, ti],
                            op0=MULT,
                            op1=MULT,
                            accum_out=s_at[:, u : u + 1],
                        )
                        nc.vector.scalar_tensor_tensor(
                            out=scr_t[:, u],
                            in0=ps_t[:, 0:D],
                            scalar=1.0,
                            in1=cu[:, ti],
                            op0=MULT,
                            op1=MULT,
                            accum_out=t_at[:, u : u + 1],
                        )
                        # pre = s*g1   (Act drains psum with per-partition scale)
                        nc.scalar.activation(
                            pre[:, u],
                            ps_s[:, D : 2 * D],
                            mybir.ActivationFunctionType.Copy,
                            bias=0.0,
                            scale=s_at[:, u : u + 1],
                        )
                        # h2 = t*g2
                        nc.scalar.activation(
                            h2[:, u],
                            ps_t[:, D : 2 * D],
                            mybir.ActivationFunctionType.Copy,
                            bias=0.0,
                            scale=t_at[:, u : u + 1],
                        )

                    # Pair-batched SBUF finish (Pool): out = pre + h2
                    # (bias add + relu happen on the host)
                    nc.gpsimd.tensor_add(ot[:, 2 * j : 2 * j + 2], pre[:], h2[:])

                nc.sync.dma_start(out_d[:, t0 : t0 + gs, :], ot[:, 0:gs])
                t0 += gs

    nc.compile()
    return nc


def _get_nc() -> bass.Bass:
    global _CACHED_NC
    if _CACHED_NC is None:
        _CACHED_NC = _build_nc()
    return _CACHED_NC


def _prep_inputs(
    cur, spatial_hyperedge_emb, temporal_hyperedge_emb,
    node_proj, spatial_edge_proj, temporal_edge_proj, theta_w, theta_b,
):
    cur = np.ascontiguousarray(cur, np.float32)
    spa = np.ascontiguousarray(spatial_hyperedge_emb, np.float32)
    tmp = np.ascontiguousarray(temporal_hyperedge_emb, np.float32)
    wn = np.asarray(node_proj, np.float64)
    ws = np.asarray(spatial_edge_proj, np.float64)
    wt = np.asarray(temporal_edge_proj, np.float64)
    th = np.asarray(theta_w, np.float64)
    b = np.asarray(theta_b, np.float32)

    w_scat = np.concatenate([(ws @ wn.T) / 16.0, ws @ th.T], axis=1)
    w_tcat = np.concatenate([(wt @ wn.T) / 16.0, wt @ th.T], axis=1)
    # [256, 512] -> [128, 2, 512]: [p, kc, f] = w[kc*128+p, f]
    w_scat = w_scat.reshape(2, 128, 2 * D).transpose(1, 0, 2).astype(BF16_NP)
    w_tcat = w_tcat.reshape(2, 128, 2 * D).transpose(1, 0, 2).astype(BF16_NP)
    pad = N_CORES * ROWS - N

    def shard(x):
        x = np.concatenate([x, np.zeros((pad, D), np.float32)], axis=0)
        return x.reshape(N_CORES, ROWS, D)

    cur_s, spa_s, tmp_s = shard(cur), shard(spa), shard(tmp)
    # k-major transpose: [ROWS, 256] -> [128, 2, ROWS]: [p, kc, n] = x[n, kc*128+p]
    spa_T = np.ascontiguousarray(
        spa_s.reshape(N_CORES, ROWS, 2, 128).transpose(0, 3, 2, 1)
    ).astype(BF16_NP)
    tmp_T = np.ascontiguousarray(
        tmp_s.reshape(N_CORES, ROWS, 2, 128).transpose(0, 3, 2, 1)
    ).astype(BF16_NP)
    # partition-major: [ROWS, 256] -> [128, TILES, 256]: [p, t, f] = x[t*128+p, f]
    cur_P = np.ascontiguousarray(
        cur_s.reshape(N_CORES, TILES, 128, D).transpose(0, 2, 1, 3)
    ).astype(BF16_NP)

    in_maps = []
    for c in range(N_CORES):
        in_maps.append(
            {
                "spaT": np.ascontiguousarray(spa_T[c]),
                "tmpT": np.ascontiguousarray(tmp_T[c]),
                "curP": np.ascontiguousarray(cur_P[c]),
                "w_scat": w_scat,
                "w_tcat": w_tcat,
            }
        )
    return in_maps


def kernel(**inputs) -> np.ndarray:
    in_maps = _prep_inputs(**inputs)
    nc = _get_nc()
    res = run_bass_kernel_spmd(nc, in_maps, list(range(N_CORES)))
    b = np.asarray(inputs["theta_b"], np.float32)
    # out: [128, TILES, 256] partition-major bf16 -> [ROWS, 256] f32;
    # bias + relu applied on host (elementwise epilogue)
    outs = [
        np.asarray(res.results[c]["out"])
        .transpose(1, 0, 2)
        .reshape(ROWS, D)
        .astype(np.float32)
        for c in range(N_CORES)
    ]
    z = np.concatenate(outs, axis=0)[:N] + b
    return np.ascontiguousarray(np.maximum(z, 0.0))
